# revision 28
# baseline (speedup 1.0000x reference)
"""MoE routing gate kernel for Trainium2 (8 NeuronCores, data-parallel).

Computes, for x[32768, 2048], weight[64, 2048], bias[64]:
    logits = x @ weight.T
    probs  = softmax(logits, axis=-1)
    idx    = top_k(probs + bias, 6).indices
    w      = take_along_axis(probs, idx)
returning (w float32 [32768, 6], idx int32 [32768, 6]).

Sharding: tokens split 4096/core across 8 cores; weight/bias replicated.

Per-core pipeline (memory-bound; HBM floor ~66us for the 24.6MB shard):
  - x streams at 3 bytes/element (fp16 hi + fp8e4m3 lo,
    lo = (x-fp16(x))*2048); three matmul passes accumulate fp32 logits:
    hi @ w_hi + hi @ w_lo + lo8 @ (w_hi/2048). |logit err| ~2.5e-5 is
    REQUIRED: top-k index flips grow ~linearly in logit error (gaps at
    the rank-6 boundary are ~1e-4) and the graded rel-err on the index
    output is quadratic in flips; 2-byte encodings measurably fail.
  - All weights ride ONE packed tensor issued on the sync ring AHEAD of
    x (FIFO): they land in ~2.5us at full stream rate. On their own
    dynamic queue they'd round-robin against x packets and finish at
    ~25us, gating the early matmul passes and stalling shared
    DMA-semaphore lanes.
  - ~24 dummy matmuls at t~7us (during the dead head before the first
    x chunk lands) trip the PE's HAM clock gate so real matmuls run at
    2.4GHz from the start instead of 1.2GHz.
  - Super-groups: 3x1024 tokens then 2x512. Matmul pairs are
    column-tiled (two half-width matmuls stream concurrently through PE
    column groups 0-63/64-127).
  - HWDGE DMA fan-out rule (probed): a [P, ...] descriptor splits into
    n = largest-divisor-of-P <= 16 consecutive partition slices,
    assigned to engines E64..E63+n in order. DMA engine 15 (E79) hosts
    the dynamic queue heads and runs ~15% slower than its peers - it is
    the stream straggler. The LAST 512-token group contracts over
    K=120 windows (120 = 15 slices of 8 -> E79 gets ZERO bytes), which
    rebalances per-engine finish times (optimum skew ~= one 512-group).
    Its final chunk is a normal K=128 one, consumed LAST: only ~200KB
    and 3 matmul passes remain after the final HBM byte, shrinking the
    serial tail.
  - Per super-group finish (deferred one sg so transposes slot into the
    next sg's DMA-wait bubbles): logits^T -> ACT copy -> PE transposes
    into two PSUM banks (bases 0/64 must not share a bank - hangs HW),
    per-j ACT exp (accum_out = row sum), DVE q = exp + sum*bias (ranks
    identically to probs + bias), Max8/MaxIndex8.
  - Output: one packed [128, nj, 15] f32 tile per sg: cols 0-7 top-8 q,
    8-13 top-6 indices (u32->f32 cast, exact for idx<64), 14 exp-sum.
    ~250KB/core vs 1.1MB for shipping all exp values; the host
    reconstructs w_k = (q_k - sum*bias[idx_k]) / sum.
"""

import numpy as np
import ml_dtypes

import concourse.bacc as bacc
import concourse.bass as bass
import concourse.mybir as mybir
import concourse.tile as tile
from concourse.bass_utils import run_bass_kernel_spmd

F32 = mybir.dt.float32
F16 = mybir.dt.float16
F8E4 = mybir.dt.float8e4
U32 = mybir.dt.uint32
OP = mybir.AluOpType
EXP = mybir.ActivationFunctionType.Exp

TOKENS, DIM, E, TOPK, NCORES = 32768, 2048, 64, 6, 8
KC = DIM // 128          # contraction chunks of 128 (a-group)
SGS_A = 3                # super-groups of 1024 tokens
SGT_A, KQ_A = 1024, 4    # chunk = [128, KQ, sgt]
SGS_B = 2                # trailing super-groups of 512 tokens
SGT_B, KQ_B = 512, 8
KB = 120                 # b1 contraction window: 120 = 15 engines x 8
                         # partition-lines -> DMA engine 15 (E79, the
                         # straggler) gets ZERO bytes for these chunks
NB = 16                  # b1 K=120 chunks; then one full K=128 chunk
PACKW = 15               # mx8 | mi6 | sum
WARMUP_MM = 24           # dummy matmuls at t~7us to trip HAM to 2.4GHz


def build_nc():
    nc = bacc.Bacc("TRN2", target_bir_lowering=False, debug=False)

    # a-group chunks: sg0 starts with two 4-k chunks (small first transfer
    # so the first matmul fires right as the PE warm-up ends), then 8-k
    # chunks (16KB hi lines, fewer descriptors -> fewer DMA-semaphore-lane
    # recycle stalls on the sync engine).
    xhi_a4 = nc.dram_tensor("xhi_a4", [2, 128, 4, SGT_A], F16, kind="ExternalInput")
    xlo_a4 = nc.dram_tensor("xlo_a4", [2, 128, 4, SGT_A], F8E4, kind="ExternalInput")
    xhi_a8 = nc.dram_tensor("xhi_a8", [5, 128, 8, SGT_A], F16, kind="ExternalInput")
    xlo_a8 = nc.dram_tensor("xlo_a8", [5, 128, 8, SGT_A], F8E4, kind="ExternalInput")
    xhi_b0 = nc.dram_tensor(
        "xhi_b0", [NB // KQ_B, 128, KQ_B, SGT_B], F16, kind="ExternalInput"
    )
    xlo_b0 = nc.dram_tensor(
        "xlo_b0", [NB // KQ_B, 128, KQ_B, SGT_B], F8E4, kind="ExternalInput"
    )
    xhi_b1 = nc.dram_tensor(
        "xhi_b1", [NB // KQ_B, KB, KQ_B, SGT_B], F16, kind="ExternalInput"
    )
    xlo_b1 = nc.dram_tensor(
        "xlo_b1", [NB // KQ_B, KB, KQ_B, SGT_B], F8E4, kind="ExternalInput"
    )
    xhi_b1f = nc.dram_tensor("xhi_b1f", [128, 1, SGT_B], F16, kind="ExternalInput")
    xlo_b1f = nc.dram_tensor("xlo_b1f", [128, 1, SGT_B], F8E4, kind="ExternalInput")
    w_all = nc.dram_tensor("w_all", [128, 3, KC, E], F16, kind="ExternalInput")
    wb_all = nc.dram_tensor("wb_all", [128, 3, NB + 1, E], F16, kind="ExternalInput")
    misc = nc.dram_tensor("misc", [128, 2, 64], F32, kind="ExternalInput")
    o_pk_a = nc.dram_tensor(
        "o_pk_a", [SGS_A, 128, SGT_A // 128, PACKW], F32, kind="ExternalOutput"
    )
    o_pk_b = nc.dram_tensor(
        "o_pk_b", [SGS_B, 128, SGT_B // 128, PACKW], F32, kind="ExternalOutput"
    )

    with tile.TileContext(nc) as tc:
        with (
            tc.tile_pool(name="consts", bufs=1) as cpool,
            tc.tile_pool(name="xha4", bufs=2) as xha4p,
            tc.tile_pool(name="xla4", bufs=2) as xla4p,
            tc.tile_pool(name="xha8", bufs=3) as xha8p,
            tc.tile_pool(name="xla8", bufs=3) as xla8p,
            tc.tile_pool(name="xb0", bufs=2) as xb0p,
            tc.tile_pool(name="xb1", bufs=2) as xb1p,
            tc.tile_pool(name="xb1f", bufs=1) as xb1fp,
            tc.tile_pool(name="lt", bufs=3) as ltp,
            tc.tile_pool(name="ex", bufs=3) as exp_,
            tc.tile_pool(name="wk", bufs=2) as wkp,
            tc.tile_pool(name="small", bufs=3) as smp,
            tc.tile_pool(name="acc", bufs=3, space="PSUM") as accp,
            tc.tile_pool(name="wup", bufs=1, space="PSUM") as wupp,
            tc.tile_pool(name="tr", bufs=2, space="PSUM") as trp,
        ):
            # PE warm-up on memset dummies; no DMA dependencies so the
            # Tensor queue runs these right after the preamble barrier.
            wdum = cpool.tile([128, 64], F16)
            nc.vector.memset(wdum, 0)
            xdum = cpool.tile([128, 512], F16)
            nc.vector.memset(xdum, 0)
            wup = wupp.tile([128, 512], F32, tag="wup")
            for _ in range(WARMUP_MM):
                nc.tensor.matmul(wup[0:64], wdum, xdum, start=True, stop=True)

            # weights FIRST on the sync ring (see module docstring)
            cw = cpool.tile([128, 3, KC, E], F16)
            nc.sync.dma_start(cw, w_all[:])
            cmisc = cpool.tile([128, 2, 64], F32)
            nc.sync.dma_start(cmisc, misc[:])
            cbias = cmisc[:, 0]
            cident = cmisc[:, 1]
            cwb = cpool.tile([128, 3, NB + 1, E], F16)

            def finish_sg(out_dram, idx, acc, sgt):
                """Transpose/softmax/rank/pack for a finished super-group."""
                nj = sgt // 128
                half = nj // 2
                grp = sgt // 2

                lt = ltp.tile([128, 512], F32, tag="lt")
                nc.scalar.copy(lt[0:64, 0:grp], acc[0:64, 0:grp])
                nc.scalar.copy(lt[64:128, 0:grp], acc[64:128, 0:grp])

                # transposes into two PSUM tiles; tiles are padded to a
                # full 2KB bank so base-0 and base-64 reads never share a
                # bank (sharing hangs the HW).
                tpsA = trp.tile([128, 8, E], F32, tag="tpsA")
                tpsB = trp.tile([128, 8, E], F32, tag="tpsB")
                for j in range(nj):
                    base = 64 * (j // half)
                    tps = tpsA if j < half else tpsB
                    nc.tensor.transpose(
                        tps[:, j % half],
                        lt[base:base + 64, (j % half) * 128:(j % half + 1) * 128],
                        cident[base:base + 64, :],
                    )

                ex = exp_.tile([128, 8, E], F32, tag="ex")
                q = wkp.tile([128, 8, E], F32, tag="q")
                pk = smp.tile([128, 8, 16], F32, tag="pk")
                mi = smp.tile([128, 8, 8], U32, tag="mi")
                for j in range(nj):
                    tps = (tpsA if j < half else tpsB)[:, j % half]
                    nc.scalar.activation(
                        ex[:, j], tps, EXP, accum_out=pk[:, j, 14:15]
                    )
                    nc.vector.scalar_tensor_tensor(
                        q[:, j], cbias, pk[:, j, 14:15], ex[:, j],
                        OP.mult, OP.add,
                    )
                    nc.vector.max(pk[:, j, 0:8], q[:, j])
                    nc.vector.max_index(mi[:, j], pk[:, j, 0:8], q[:, j])
                    # u32 -> f32 value cast; exact for idx < 64
                    nc.vector.tensor_copy(pk[:, j, 8:14], mi[:, j, 0:TOPK])

                nc.gpsimd.dma_start(out_dram[idx], pk[:, 0:nj, 0:PACKW])

            pending = None  # (out_dram, idx, acc, sgt) awaiting finish

            def run_sg(out_dram, idx, sgt, chunks, wtile, ksizes):
                """chunks: list of (hi_tile, lo_tile, nk); ksizes: K per
                global chunk index."""
                nonlocal pending
                grp = sgt // 2
                acc = accp.tile([128, 512], F32)
                nchunks = len(ksizes)
                kk = 0
                first_nk = chunks[0][2]
                for th, tl, nk in chunks:
                    for s in range(nk):
                        K = ksizes[kk]
                        hi_k = th[0:K, s]   # [K, sgt] fp16
                        lo_k = tl[0:K, s]   # [K, sgt] fp8
                        for p in range(3):
                            w = wtile[0:K, p, kk, :]
                            xs = (hi_k, hi_k, lo_k)[p]
                            first = kk == 0 and p == 0
                            last = kk == nchunks - 1 and p == 2
                            nc.tensor.matmul(
                                acc[0:64, 0:grp], w, xs[:, 0:grp],
                                start=first, stop=last, tile_position=(0, 0),
                            )
                            nc.tensor.matmul(
                                acc[64:128, 0:grp], w, xs[:, grp:sgt],
                                start=first, stop=last, tile_position=(0, 64),
                                skip_group_check=True,
                            )
                        kk += 1
                        if kk == first_nk and pending is not None:
                            # issue previous sg's finish after this sg's
                            # first chunk: the in-order Tensor queue then
                            # runs its transposes inside DMA-wait bubbles
                            finish_sg(*pending)
                            pending = None
                pending = (out_dram, idx, acc, sgt)

            n4 = n8 = 0

            def a_chunk(kq):
                nonlocal n4, n8
                if kq == 4:
                    th = xha4p.tile([128, 4, SGT_A], F16, tag="xh4")
                    nc.sync.dma_start(th, xhi_a4[n4])
                    tl = xla4p.tile([128, 4, SGT_A], F8E4, tag="xl4")
                    nc.sync.dma_start(tl, xlo_a4[n4])
                    n4 += 1
                else:
                    th = xha8p.tile([128, 8, SGT_A], F16, tag="xh8")
                    nc.sync.dma_start(th, xhi_a8[n8])
                    tl = xla8p.tile([128, 8, SGT_A], F8E4, tag="xl8")
                    nc.sync.dma_start(tl, xlo_a8[n8])
                    n8 += 1
                return (th, tl, kq)

            for i in range(SGS_A):
                kqs = [4, 4, 8] if i == 0 else [8, 8]
                chunks = []
                for c, kq in enumerate(kqs):
                    chunks.append(a_chunk(kq))
                    if i == 0 and c == 0:
                        # b-group weights ride in-stream, needed only late
                        nc.sync.dma_start(cwb, wb_all[:])
                run_sg(o_pk_a, i, SGT_A, chunks, cw, [128] * KC)

            # b0: normal K=128 chunking, reuses the a-group weight tiling
            chunks = []
            for c in range(NB // KQ_B):
                th = xb0p.tile([128, KQ_B, SGT_B], F16, tag="xh")
                nc.sync.dma_start(th, xhi_b0[c])
                tl = xb0p.tile([128, KQ_B, SGT_B], F8E4, tag="xl")
                nc.sync.dma_start(tl, xlo_b0[c])
                chunks.append((th, tl, KQ_B))
            run_sg(o_pk_b, 0, SGT_B, chunks, cw, [128] * KC)

            # b1 (last): K=120 chunks (E79-skewed) + one K=128 tail chunk.
            # The tail chunk is tiny AND issued early (it is fully
            # buffered), so the last-arriving bytes are the 15-engine K=120
            # groups and only ~3 matmul passes + the finish chain remain
            # after the final HBM byte.
            thf = xb1fp.tile([128, 1, SGT_B], F16, tag="xhf")
            nc.sync.dma_start(thf, xhi_b1f[:])
            tlf = xb1fp.tile([128, 1, SGT_B], F8E4, tag="xlf")
            nc.sync.dma_start(tlf, xlo_b1f[:])
            chunks = []
            for c in range(NB // KQ_B):
                th = xb1p.tile([KB, KQ_B, SGT_B], F16, tag="xh")
                nc.sync.dma_start(th, xhi_b1[c])
                tl = xb1p.tile([KB, KQ_B, SGT_B], F8E4, tag="xl")
                nc.sync.dma_start(tl, xlo_b1[c])
                chunks.append((th, tl, KQ_B))
            chunks.append((thf, tlf, 1))
            run_sg(o_pk_b, 1, SGT_B, chunks, cwb, [KB] * NB + [128])

            finish_sg(*pending)
    return nc


_CACHE = {}


def _get_compiled():
    if "nc" not in _CACHE:
        nc = build_nc()
        nc.compile()
        _CACHE["nc"] = nc
    return _CACHE["nc"]


def _prep_shared(weight, bias):
    w = np.asarray(weight, np.float32)
    w_hi = w.astype(np.float16)
    w_lo = (w - w_hi.astype(np.float32)).astype(np.float16)
    w_3 = (w_hi.astype(np.float32) * (1.0 / 2048.0)).astype(np.float16)

    def wtile(a):  # [E, DIM] -> [128, KC, E]
        return np.ascontiguousarray(a.T).reshape(KC, 128, E).transpose(1, 0, 2)

    def wbtile(a):  # [E, DIM] -> [128, NB+1, E] with K=120 chunking
        aT = np.ascontiguousarray(a.T)  # [DIM, E]
        out = np.zeros((128, NB + 1, E), a.dtype)
        for c in range(NB):
            out[0:KB, c] = aT[c * KB:(c + 1) * KB]
        out[:, NB] = aT[NB * KB:DIM]
        return out

    w_all = np.ascontiguousarray(
        np.stack([wtile(v) for v in (w_hi, w_lo, w_3)], axis=1)
    )
    wb_all = np.ascontiguousarray(
        np.stack([wbtile(v) for v in (w_hi, w_lo, w_3)], axis=1)
    )
    misc = np.empty((128, 2, 64), np.float32)
    misc[:, 0] = np.asarray(bias, np.float32)
    misc[:, 1] = np.tile(np.eye(64, dtype=np.float32), (2, 1))
    return {"w_all": w_all, "wb_all": wb_all, "misc": np.ascontiguousarray(misc)}


def prep_core_inputs(x, weight, bias, ncores=NCORES):
    shared = _prep_shared(weight, bias)
    x = np.asarray(x, np.float32)
    tpc = x.shape[0] // ncores
    na = SGS_A * SGT_A
    # whole-tensor transpose + casts once (not per core)
    xT = np.ascontiguousarray(x.T)           # [DIM, TOKENS]
    xhT = xT.astype(np.float16)
    xlT = ((xT - xhT.astype(np.float32)) * 2048.0).astype(
        ml_dtypes.float8_e4m3fn
    )
    del xT

    def chunk_a(xx, d0, nk, t):  # dims [d0, d0+nk*128) of sg t -> [128,nk,1024]
        sl = xx[d0:d0 + nk * 128, t * SGT_A:(t + 1) * SGT_A]
        return sl.reshape(nk, 128, SGT_A).transpose(1, 0, 2)

    def pack_a4(xx):  # sg0 chunks 0,1 (4-k each)
        return np.ascontiguousarray(
            np.stack([chunk_a(xx, 0, 4, 0), chunk_a(xx, 512, 4, 0)])
        )

    def pack_a8(xx):  # sg0 chunk 2 + sg1/sg2 chunks (8-k each)
        return np.ascontiguousarray(np.stack([
            chunk_a(xx, 1024, 8, 0),
            chunk_a(xx, 0, 8, 1), chunk_a(xx, 1024, 8, 1),
            chunk_a(xx, 0, 8, 2), chunk_a(xx, 1024, 8, 2),
        ]))

    def pack_b0(xx):  # [DIM, 512] -> [2, 128, 8, 512], K=128 chunks
        x4 = xx.reshape(NB // KQ_B, KQ_B, 128, SGT_B)
        return np.ascontiguousarray(x4.transpose(0, 2, 1, 3))

    def pack_b1(xx):  # [DIM, 512] main -> [2, 120, 8, 512], K=120 chunks
        x4 = xx[0:NB * KB].reshape(NB // KQ_B, KQ_B, KB, SGT_B)
        return np.ascontiguousarray(x4.transpose(0, 2, 1, 3))

    def pack_b1f(xx):  # remainder dims 1920:2048 -> [128, 1, 512]
        return np.ascontiguousarray(xx[NB * KB:DIM])[:, None, :]

    in_maps = []
    for c in range(ncores):
        lo = c * tpc
        ah, al = xhT[:, lo:lo + na], xlT[:, lo:lo + na]
        b0h = xhT[:, lo + na:lo + na + SGT_B]
        b0l = xlT[:, lo + na:lo + na + SGT_B]
        b1h = xhT[:, lo + na + SGT_B:lo + tpc]
        b1l = xlT[:, lo + na + SGT_B:lo + tpc]
        in_maps.append({
            "xhi_a4": pack_a4(ah), "xlo_a4": pack_a4(al),
            "xhi_a8": pack_a8(ah), "xlo_a8": pack_a8(al),
            "xhi_b0": pack_b0(b0h), "xlo_b0": pack_b0(b0l),
            "xhi_b1": pack_b1(b1h), "xlo_b1": pack_b1(b1l),
            "xhi_b1f": pack_b1f(b1h), "xlo_b1f": pack_b1f(b1l),
            **shared,
        })
    return in_maps


def unpack_outputs(res_list, bias):
    bias = np.asarray(bias, np.float64)
    ws, idxs = [], []
    for r in res_list:
        for nm in ("o_pk_a", "o_pk_b"):
            pk = np.asarray(r[nm], np.float64)  # [nsg, 128, nj, 15]
            # token t = sg*sgt + 128*j + p
            pk = pk.transpose(0, 2, 1, 3).reshape(-1, PACKW)
            mx = pk[:, 0:TOPK]
            mi = np.rint(pk[:, 8:14]).astype(np.int64)
            ssum = pk[:, 14:15]
            wv = (mx - ssum * bias[mi]) / ssum
            ws.append(wv)
            idxs.append(mi)
    return (
        np.ascontiguousarray(np.concatenate(ws)).astype(np.float32),
        np.ascontiguousarray(np.concatenate(idxs)).astype(np.int32),
    )


def run(x, weight, bias, trace=False, **kwargs):
    x = np.asarray(x, np.float32)
    nc = _get_compiled()
    in_maps = prep_core_inputs(x, weight, bias)
    res = run_bass_kernel_spmd(
        nc, in_maps, list(range(NCORES)), trace=trace, **kwargs
    )
    w, i = unpack_outputs(res.results, bias)
    return w, i, res


def kernel(x, weight, bias):
    w, i, _ = run(x, weight, bias, trace=False)
    return w, i


# revision 29
# speedup vs baseline: 1.0517x; 1.0517x over previous
"""MoE routing gate kernel for Trainium2 (8 NeuronCores, data-parallel).

Computes, for x[32768, 2048], weight[64, 2048], bias[64]:
    logits = x @ weight.T
    probs  = softmax(logits, axis=-1)
    idx    = top_k(probs + bias, 6).indices
    w      = take_along_axis(probs, idx)
returning (w float32 [32768, 6], idx int32 [32768, 6]).

Sharding: tokens split 4096/core across 8 cores; weight/bias replicated.

Per-core pipeline (memory-bound; HBM floor ~66us for the 24.6MB shard):
  - x streams at 3 bytes/element (fp16 hi + fp8e4m3 lo with
    lo = (x-fp16(x))*2048); three matmul passes accumulate fp32 logits
    in PSUM: hi @ w_hi + hi @ w_lo + lo8 @ (w_hi/2048). |logit err|
    ~2.5e-5 is REQUIRED: top-k index flips grow ~linearly in logit
    error (biased-score gaps at the rank-6 boundary are ~1e-4) and the
    graded rel-err on the index output is quadratic in flips; 2-byte
    encodings measurably fail (fp16-only: 430 flips, rel_i 3.5e-2).
  - 4 super-groups of 1024 tokens. Matmul pairs are column-tiled:
    group g=0 lands in PE columns 0-63 (PSUM partitions 0-63), g=1 in
    columns 64-127, so two N=512 matmuls stream concurrently.
  - All weights ride ONE packed tensor issued on the sync ring AHEAD
    of x (FIFO -> lands in ~2.5us at full stream rate). On the scalar
    ring's dynamic queue they round-robin against x packets and only
    finish at ~25us, gating the pass-2/3 matmuls of sg0.
  - ~24 dummy matmuls during the dead head (t~7-13us, before the first
    x chunk lands) trip the PE's HAM clock gate so real matmuls run at
    2.4GHz from the start instead of 1.2GHz.
  - HWDGE DMA fan-out rule (probed): a [P, ...] descriptor splits into
    n = largest-divisor-of-P <= 16 consecutive partition slices on
    engines E64..E63+n. Engine 15 (E79) hosts the dynamic queue heads
    and runs ~15% slower - it is the stream straggler. The LAST
    super-group contracts over K=120 windows (120 = 15 x 8 -> E79 gets
    ZERO bytes there), rebalancing per-engine finish times. Its
    remainder K=128 chunk is issued EARLY (fully buffered) but
    consumed LAST, so only ~3 matmul passes + the finish chain follow
    the final HBM byte.
  - The last sg's chunks live in dedicated, fully-buffered SBUF pools
    so its DMA issue is never gated on the PE freeing earlier buffers.
  - Per super-group finish (deferred one sg behind the matmul issue so
    the in-order Tensor queue runs its transposes inside DMA-wait
    bubbles): logits^T -> ACT copy -> 8 PE transposes into two PSUM
    tiles (transposes reading partition bases 0 and 64 must not share
    a PSUM bank - sharing hangs the HW); per-j ACT exp with
    accum_out = row sum; DVE q = exp + sum*bias (same ordering as
    probs + bias); Max8/MaxIndex8.
  - Output: one packed [128, nj, 15] f32 tile per sg: cols 0-7 = top-8
    q values, 8-13 = top-6 indices (u32->f32 cast, exact for idx<64),
    14 = exp row-sum. ~250KB/core instead of shipping all 64 exp
    values (1.1MB); the host reconstructs
    w_k = (q_k - sum*bias[idx_k]) / sum.
"""

import numpy as np
import ml_dtypes

import concourse.bacc as bacc
import concourse.bass as bass
import concourse.mybir as mybir
import concourse.tile as tile
from concourse.bass_utils import run_bass_kernel_spmd

F32 = mybir.dt.float32
F16 = mybir.dt.float16
F8E4 = mybir.dt.float8e4
U32 = mybir.dt.uint32
OP = mybir.AluOpType
EXP = mybir.ActivationFunctionType.Exp

TOKENS, DIM, E, TOPK, NCORES = 32768, 2048, 64, 6, 8
KC = DIM // 128     # contraction chunks of 128
SGT = 1024          # tokens per super-group
NSG = 4             # super-groups per core
KQ = 4              # k-slices per DMA chunk
KB = 120            # last-sg contraction window (E79 skew): 15 engines x 8
NB = 16             # last-sg K=120 chunks; then one K=128 remainder chunk
NJ = SGT // 128
PACKW = 15          # mx8 | mi6 | sum
WARMUP_MM = 24


def build_nc():
    nc = bacc.Bacc("TRN2", target_bir_lowering=False, debug=False)

    xhi = nc.dram_tensor(
        "xhi", [NSG - 1, KC // KQ, 128, KQ, SGT], F16, kind="ExternalInput"
    )
    xlo = nc.dram_tensor(
        "xlo", [NSG - 1, KC // KQ, 128, KQ, SGT], F8E4, kind="ExternalInput"
    )
    xhi_s = nc.dram_tensor(
        "xhi_s", [NB // KQ, KB, KQ, SGT], F16, kind="ExternalInput"
    )
    xlo_s = nc.dram_tensor(
        "xlo_s", [NB // KQ, KB, KQ, SGT], F8E4, kind="ExternalInput"
    )
    xhi_f = nc.dram_tensor("xhi_f", [128, 1, SGT], F16, kind="ExternalInput")
    xlo_f = nc.dram_tensor("xlo_f", [128, 1, SGT], F8E4, kind="ExternalInput")
    w_all = nc.dram_tensor("w_all", [128, 3, KC, E], F16, kind="ExternalInput")
    wb_all = nc.dram_tensor("wb_all", [128, 3, NB + 1, E], F16, kind="ExternalInput")
    misc = nc.dram_tensor("misc", [128, 2, 64], F32, kind="ExternalInput")
    o_pk = nc.dram_tensor(
        "o_pk", [NSG, 128, NJ, PACKW], F32, kind="ExternalOutput"
    )

    with tile.TileContext(nc) as tc:
        with (
            tc.tile_pool(name="consts", bufs=1) as cpool,
            tc.tile_pool(name="xh", bufs=8) as xhp,
            tc.tile_pool(name="xl", bufs=8) as xlp,
            tc.tile_pool(name="xh3", bufs=4) as xh3p,
            tc.tile_pool(name="xl3", bufs=4) as xl3p,
            tc.tile_pool(name="xf", bufs=1) as xfp,
            tc.tile_pool(name="lt", bufs=3) as ltp,
            tc.tile_pool(name="ex", bufs=3) as exp_,
            tc.tile_pool(name="wk", bufs=2) as wkp,
            tc.tile_pool(name="small", bufs=3) as smp,
            tc.tile_pool(name="acc", bufs=3, space="PSUM") as accp,
            tc.tile_pool(name="wup", bufs=1, space="PSUM") as wupp,
            tc.tile_pool(name="tr", bufs=2, space="PSUM") as trp,
        ):
            # PE warm-up on memset dummies (no DMA deps; runs right after
            # the preamble barrier while the first x chunk is in flight)
            wdum = cpool.tile([128, 64], F16)
            nc.vector.memset(wdum, 0)
            xdum = cpool.tile([128, 512], F16)
            nc.vector.memset(xdum, 0)
            wup = wupp.tile([128, 512], F32, tag="wup")
            for _ in range(WARMUP_MM):
                nc.tensor.matmul(wup[0:64], wdum, xdum, start=True, stop=True)

            # weights first on the sync ring
            cw = cpool.tile([128, 3, KC, E], F16)
            nc.sync.dma_start(cw, w_all[:])
            cmisc = cpool.tile([128, 2, 64], F32)
            nc.sync.dma_start(cmisc, misc[:])
            cbias = cmisc[:, 0]
            cident = cmisc[:, 1]
            cwb = cpool.tile([128, 3, NB + 1, E], F16)

            def finish_sg(idx, acc):
                lt = ltp.tile([128, 512], F32, tag="lt")
                nc.scalar.copy(lt[0:64], acc[0:64])
                nc.scalar.copy(lt[64:128], acc[64:128])

                tpsA = trp.tile([128, NJ // 2, E], F32, tag="tpsA")
                tpsB = trp.tile([128, NJ // 2, E], F32, tag="tpsB")
                for j in range(NJ):
                    base = 64 * (j // (NJ // 2))
                    tps = tpsA if j < NJ // 2 else tpsB
                    nc.tensor.transpose(
                        tps[:, j % (NJ // 2)],
                        lt[base:base + 64, (j % 4) * 128:(j % 4 + 1) * 128],
                        cident[base:base + 64, :],
                    )

                ex = exp_.tile([128, NJ, E], F32, tag="ex")
                q = wkp.tile([128, NJ, E], F32, tag="q")
                pk = smp.tile([128, NJ, 16], F32, tag="pk")
                mi = smp.tile([128, NJ, 8], U32, tag="mi")
                for j in range(NJ):
                    tps = (tpsA if j < NJ // 2 else tpsB)[:, j % (NJ // 2)]
                    nc.scalar.activation(
                        ex[:, j], tps, EXP, accum_out=pk[:, j, 14:15]
                    )
                    nc.vector.scalar_tensor_tensor(
                        q[:, j], cbias, pk[:, j, 14:15], ex[:, j],
                        OP.mult, OP.add,
                    )
                    nc.vector.max(pk[:, j, 0:8], q[:, j])
                    nc.vector.max_index(mi[:, j], pk[:, j, 0:8], q[:, j])
                    nc.vector.tensor_copy(pk[:, j, 8:14], mi[:, j, 0:TOPK])

                nc.gpsimd.dma_start(o_pk[idx], pk[:, :, 0:PACKW])

            pending = None  # (sg, acc) awaiting finish

            def mm_sg(sg, chunks, wtile, ksizes):
                """chunks: [(hi_tile, lo_tile, nk)]; ksizes: K per chunk."""
                nonlocal pending
                acc = accp.tile([128, SGT // 2], F32)
                grp = SGT // 2
                first_nk = chunks[0][2]
                nchunks = len(ksizes)
                kk = 0
                for th, tl, nk in chunks:
                    for s in range(nk):
                        K = ksizes[kk]
                        hi_k, lo_k = th[0:K, s], tl[0:K, s]
                        for p in range(3):
                            w = wtile[0:K, p, kk, :]
                            xs = (hi_k, hi_k, lo_k)[p]
                            first = kk == 0 and p == 0
                            last = kk == nchunks - 1 and p == 2
                            nc.tensor.matmul(
                                acc[0:64], w, xs[:, 0:grp],
                                start=first, stop=last, tile_position=(0, 0),
                            )
                            nc.tensor.matmul(
                                acc[64:128], w, xs[:, grp:SGT],
                                start=first, stop=last, tile_position=(0, 64),
                                skip_group_check=True,
                            )
                        kk += 1
                        if kk == first_nk and pending is not None:
                            finish_sg(*pending)
                            pending = None
                pending = (sg, acc)

            for sg in range(NSG - 1):
                chunks = []
                for c in range(KC // KQ):
                    th = xhp.tile([128, KQ, SGT], F16, tag="xh")
                    nc.sync.dma_start(th, xhi[sg, c])
                    chunks.append(th)
                    tl = xlp.tile([128, KQ, SGT], F8E4, tag="xl")
                    nc.sync.dma_start(tl, xlo[sg, c])
                    chunks[-1] = (th, tl, KQ)
                    if sg == 0 and c == 0:
                        # last-sg weights + its remainder chunk ride
                        # in-stream early (fully buffered, consumed last)
                        nc.sync.dma_start(cwb, wb_all[:])
                        thf = xfp.tile([128, 1, SGT], F16, tag="xhf")
                        nc.sync.dma_start(thf, xhi_f[:])
                        tlf = xfp.tile([128, 1, SGT], F8E4, tag="xlf")
                        nc.sync.dma_start(tlf, xlo_f[:])
                mm_sg(sg, chunks, cw, [128] * KC)

            # last sg: K=120 chunks (E79 skew), K=128 remainder consumed last
            chunks = []
            for c in range(NB // KQ):
                th = xh3p.tile([KB, KQ, SGT], F16, tag="xh")
                nc.sync.dma_start(th, xhi_s[c])
                tl = xl3p.tile([KB, KQ, SGT], F8E4, tag="xl")
                nc.sync.dma_start(tl, xlo_s[c])
                chunks.append((th, tl, KQ))
            chunks.append((thf, tlf, 1))
            mm_sg(NSG - 1, chunks, cwb, [KB] * NB + [128])

            finish_sg(*pending)
    return nc


_CACHE = {}


def _get_compiled():
    if "nc" not in _CACHE:
        nc = build_nc()
        nc.compile()
        _CACHE["nc"] = nc
    return _CACHE["nc"]


def _prep_shared(weight, bias):
    w = np.asarray(weight, np.float32)
    w_hi = w.astype(np.float16)
    w_lo = (w - w_hi.astype(np.float32)).astype(np.float16)
    w_3 = (w_hi.astype(np.float32) * (1.0 / 2048.0)).astype(np.float16)

    def wtile(a):  # [E, DIM] -> [128, KC, E]
        return np.ascontiguousarray(a.T).reshape(KC, 128, E).transpose(1, 0, 2)

    def wbtile(a):  # [E, DIM] -> [128, NB+1, E] with K=120 chunking
        aT = np.ascontiguousarray(a.T)  # [DIM, E]
        out = np.zeros((128, NB + 1, E), a.dtype)
        for c in range(NB):
            out[0:KB, c] = aT[c * KB:(c + 1) * KB]
        out[:, NB] = aT[NB * KB:DIM]
        return out

    w_all = np.ascontiguousarray(
        np.stack([wtile(v) for v in (w_hi, w_lo, w_3)], axis=1)
    )
    wb_all = np.ascontiguousarray(
        np.stack([wbtile(v) for v in (w_hi, w_lo, w_3)], axis=1)
    )
    misc = np.empty((128, 2, 64), np.float32)
    misc[:, 0] = np.asarray(bias, np.float32)
    misc[:, 1] = np.tile(np.eye(64, dtype=np.float32), (2, 1))
    return {"w_all": w_all, "wb_all": wb_all, "misc": np.ascontiguousarray(misc)}


def prep_core_inputs(x, weight, bias, ncores=NCORES):
    shared = _prep_shared(weight, bias)
    x = np.asarray(x, np.float32)
    tpc = x.shape[0] // ncores
    na = (NSG - 1) * SGT
    xT = np.ascontiguousarray(x.T)           # [DIM, TOKENS]
    xhT = xT.astype(np.float16)
    xlT = ((xT - xhT.astype(np.float32)) * 2048.0).astype(
        ml_dtypes.float8_e4m3fn
    )
    del xT

    def pack_a(xx):  # [DIM, 3*1024] -> [3, KC//KQ, 128, KQ, SGT]
        x6 = xx.reshape(KC // KQ, KQ, 128, NSG - 1, SGT)
        return np.ascontiguousarray(x6.transpose(3, 0, 2, 1, 4))

    def pack_s(xx):  # [DIM, 1024] main -> [4, 120, 4, 1024]
        x4 = xx[0:NB * KB].reshape(NB // KQ, KQ, KB, SGT)
        return np.ascontiguousarray(x4.transpose(0, 2, 1, 3))

    def pack_f(xx):  # remainder dims 1920:2048 -> [128, 1, 1024]
        return np.ascontiguousarray(xx[NB * KB:DIM])[:, None, :]

    in_maps = []
    for c in range(ncores):
        lo = c * tpc
        ah, al = xhT[:, lo:lo + na], xlT[:, lo:lo + na]
        sh, sl_ = xhT[:, lo + na:lo + tpc], xlT[:, lo + na:lo + tpc]
        in_maps.append({
            "xhi": pack_a(ah), "xlo": pack_a(al),
            "xhi_s": pack_s(sh), "xlo_s": pack_s(sl_),
            "xhi_f": pack_f(sh), "xlo_f": pack_f(sl_),
            **shared,
        })
    return in_maps


def unpack_outputs(res_list, bias):
    bias = np.asarray(bias, np.float64)
    ws, idxs = [], []
    for r in res_list:
        pk = np.asarray(r["o_pk"], np.float64)  # [NSG, 128, NJ, 15]
        # token t = sg*SGT + 128*j + p
        pk = pk.transpose(0, 2, 1, 3).reshape(-1, PACKW)
        mx = pk[:, 0:TOPK]
        mi = np.rint(pk[:, 8:14]).astype(np.int64)
        ssum = pk[:, 14:15]
        ws.append((mx - ssum * bias[mi]) / ssum)
        idxs.append(mi)
    return (
        np.ascontiguousarray(np.concatenate(ws)).astype(np.float32),
        np.ascontiguousarray(np.concatenate(idxs)).astype(np.int32),
    )


def run(x, weight, bias, trace=False, **kwargs):
    x = np.asarray(x, np.float32)
    nc = _get_compiled()
    in_maps = prep_core_inputs(x, weight, bias)
    res = run_bass_kernel_spmd(
        nc, in_maps, list(range(NCORES)), trace=trace, **kwargs
    )
    w, i = unpack_outputs(res.results, bias)
    return w, i, res


def kernel(x, weight, bias):
    w, i, _ = run(x, weight, bias, trace=False)
    return w, i


# revision 30
# speedup vs baseline: 1.1356x; 1.0797x over previous
"""MoE routing gate kernel for Trainium2 (8 NeuronCores, data-parallel).

Computes, for x[32768, 2048], weight[64, 2048], bias[64]:
    logits = x @ weight.T
    probs  = softmax(logits, axis=-1)
    idx    = top_k(probs + bias, 6).indices
    w      = take_along_axis(probs, idx)
returning (w float32 [32768, 6], idx int32 [32768, 6]).

Sharding: tokens split 4096/core across 8 cores; weight/bias replicated.
DMA: hi and lo chunks ride the sync ring interleaved in consumption order;
the last super-group uses dedicated SBUF tiles so its fetch is never gated
on the PE freeing earlier buffers.

Per-core pipeline (memory-bound; HBM floor ~60us for the 25MB shard):
  - x is streamed at 3 bytes/element: fp16 hi + fp8e4m3 lo with
    lo = (x - fp16(x)) * 2048 (quantized into e4m3's normal range).
    Three matmul passes accumulate fp32 logits in PSUM:
      hi @ w_hi(fp16) + hi @ w_lo(fp16) + lo8 @ w3(fp16, = w_hi/2048)
    giving |logit err| ~2.5e-5 (equivalent to the fp32 reference for
    top-k stability) while cutting HBM read traffic 25% vs fp32.
  - Matmul pairs are column-tiled: group g=0 lands in PE columns 0-63
    (PSUM partitions 0-63), g=1 in columns 64-127, so two N=512 matmuls
    stream concurrently through disjoint column groups of the array.
  - logits^T -> ACT copy to SBUF -> 8 PE transposes (identity matmul)
    into two PSUM tiles per super-group (transposes reading partition
    bases 0 and 64 must not share a PSUM bank - that hangs the HW).
  - Softmax without max-subtraction (|logits| < ~7, exp is safe in
    fp32): per-j ACT exp emits the row sum via accum_out; DVE ranks
    q = exp + sum*bias (same ordering as probs + bias) with
    Max8/MaxIndex8 and only the top-6 INDICES leave the device.
  - The exp values themselves are DMA'd out raw (1MB/core, ~2% extra
    HBM traffic); the host computes probs = exp/sum and gathers the
    top-6 weights during unpacking. This removes the expensive
    on-device one-hot gather (was ~40us of DVE time) entirely.
"""

import numpy as np
import ml_dtypes

import concourse.bacc as bacc
import concourse.bass as bass
import concourse.mybir as mybir
import concourse.tile as tile
from concourse.bass_utils import run_bass_kernel_spmd

F32 = mybir.dt.float32
F16 = mybir.dt.float16
F8E4 = mybir.dt.float8e4
I32 = mybir.dt.int32
U32 = mybir.dt.uint32
AX = mybir.AxisListType
OP = mybir.AluOpType
EXP = mybir.ActivationFunctionType.Exp

TOKENS, DIM, E, TOPK, NCORES = 32768, 2048, 64, 6, 8
KC = DIM // 128  # contraction chunks of 128
KQ = 4           # k-chunks per DMA


def build_nc(tpc, sg_t=1024):
    """Build the per-core Bass program for a tpc-token shard."""
    grp = sg_t // 2         # tokens per matmul (N), two col-tiled groups per sg
    assert grp == 512
    nsg = tpc // sg_t
    nj = sg_t // 128        # 128-token tiles per super-group
    cols = nj * TOPK        # staging cols per sg

    nc = bacc.Bacc("TRN2", target_bir_lowering=False, debug=False)

    xhi = nc.dram_tensor(
        "xhi", [nsg, KC // KQ, 128, KQ, sg_t], F16, kind="ExternalInput"
    )
    xlo = nc.dram_tensor(
        "xlo", [nsg, KC // KQ, 128, KQ, sg_t], F8E4, kind="ExternalInput"
    )
    wt_hi = nc.dram_tensor("wt_hi", [128, KC, E], F16, kind="ExternalInput")
    wt_lo = nc.dram_tensor("wt_lo", [128, KC, E], F16, kind="ExternalInput")
    wt_3 = nc.dram_tensor("wt_3", [128, KC, E], F16, kind="ExternalInput")
    bias_b = nc.dram_tensor("bias_b", [128, E], F32, kind="ExternalInput")
    ident2 = nc.dram_tensor("ident2", [128, 64], F32, kind="ExternalInput")
    ex_out = nc.dram_tensor("ex_out", [nsg, 128, nj, E], F32, kind="ExternalOutput")
    i_out = nc.dram_tensor("i_out", [nsg, 128, cols], I32, kind="ExternalOutput")

    with tile.TileContext(nc) as tc:
        with (
            tc.tile_pool(name="consts", bufs=1) as cpool,
            tc.tile_pool(name="xh", bufs=8) as xhp,
            tc.tile_pool(name="xl", bufs=8) as xlp,
            tc.tile_pool(name="xh3", bufs=4) as xh3p,
            tc.tile_pool(name="xl3", bufs=4) as xl3p,
            tc.tile_pool(name="lt", bufs=3) as ltp,
            tc.tile_pool(name="ex", bufs=3) as exp_,
            tc.tile_pool(name="wk", bufs=2) as wkp,
            tc.tile_pool(name="small", bufs=3) as smp,
            tc.tile_pool(name="stage", bufs=3) as stp,
            tc.tile_pool(name="acc", bufs=3, space="PSUM") as accp,
            tc.tile_pool(name="tr", bufs=2, space="PSUM") as trp,
        ):
            cwh = cpool.tile([128, KC, E], F16)
            nc.scalar.dma_start(cwh, wt_hi[:])
            cwl = cpool.tile([128, KC, E], F16)
            nc.scalar.dma_start(cwl, wt_lo[:])
            cw3 = cpool.tile([128, KC, E], F16)
            nc.scalar.dma_start(cw3, wt_3[:])
            cbias = cpool.tile([128, E], F32)
            nc.scalar.dma_start(cbias, bias_b[:])
            cident = cpool.tile([128, 64], F32)
            nc.scalar.dma_start(cident, ident2[:])

            def finish_sg(sg, acc):
                """Copy/transpose/softmax/rank/out for a finished super-group.

                Deferred one sg behind the matmul issue so the Tensor queue
                always has the next sg's matmuls ahead of these transposes
                (which wait on the ACT copy) - avoids a cross-engine convoy.
                """
                lt = ltp.tile([128, grp], F32)
                nc.scalar.copy(lt[0:64], acc[0:64])
                nc.scalar.copy(lt[64:128], acc[64:128])

                # 8 transposes into two PSUM tiles [128 tok, nj/2, 64 exp].
                # NB: transposes reading partition bases 0 and 64 must land in
                # different PSUM banks - mixing them in one bank hangs the HW.
                tpsA = trp.tile([128, nj // 2, E], F32, tag="tpsA")
                tpsB = trp.tile([128, nj // 2, E], F32, tag="tpsB")
                for j in range(nj):
                    base = 64 * (j // 4)
                    tps = tpsA if j < 4 else tpsB
                    nc.tensor.transpose(
                        tps[:, j % 4],
                        lt[base:base + 64, (j % 4) * 128:(j % 4 + 1) * 128],
                        cident[base:base + 64, :],
                    )

                # per-j ACT exp (accum_out = row sum); q = exp + sum*bias on
                # DVE ranks identically to probs + bias
                ex = exp_.tile([128, nj, E], F32, tag="ex")
                ssum = smp.tile([128, nj], F32, tag="ssum")
                q = wkp.tile([128, nj, E], F32, tag="q")
                mx = smp.tile([128, nj, 8], F32, tag="mx")
                mi = smp.tile([128, nj, 8], U32, tag="mi")
                for j in range(nj):
                    tps = (tpsA if j < 4 else tpsB)[:, j % 4]
                    nc.scalar.activation(
                        ex[:, j], tps, EXP, accum_out=ssum[:, j:j + 1]
                    )
                    nc.vector.scalar_tensor_tensor(
                        q[:, j], cbias, ssum[:, j:j + 1], ex[:, j],
                        OP.mult, OP.add,
                    )
                    nc.vector.max(mx[:, j], q[:, j])
                    nc.vector.max_index(mi[:, j], mx[:, j], q[:, j])

                si = stp.tile([128, nj, TOPK], I32, tag="si")
                nc.vector.tensor_copy(si, mi[:, :, 0:TOPK])

                nc.gpsimd.dma_start(ex_out[sg], ex)
                nc.gpsimd.dma_start(i_out[sg], si.rearrange("p a b -> p (a b)"))

            pending = None  # (sg, acc) awaiting finish
            for sg in range(nsg):
                # x super-group: KQ-chunk DMAs, hi+lo interleaved on sync.
                # The last sg gets DEDICATED tiles: with shared pools its
                # DMA issue is gated on the PE freeing sg0's buffers, which
                # stalls the end of the stream whenever compute runs slow.
                hp = xh3p if sg == nsg - 1 else xhp
                lp = xl3p if sg == nsg - 1 else xlp
                xh, xl = [], []
                for kq in range(KC // KQ):
                    th = hp.tile([128, KQ, sg_t], F16, tag="xh")
                    nc.sync.dma_start(th, xhi[sg, kq])
                    xh.append(th)
                    tl = lp.tile([128, KQ, sg_t], F8E4, tag="xl")
                    # lo rides the sync ring right behind its hi chunk: queue
                    # order then matches consumption order exactly (on the
                    # scalar ring, lo issue sits behind earlier sgs' exp
                    # chains and lands after future sgs' hi bytes)
                    nc.sync.dma_start(tl, xlo[sg, kq])
                    xl.append(tl)

                # 96 matmuls: col-tiled pairs (g=0 -> cols 0-63, g=1 -> 64-127).
                # The previous sg's finish-phase instructions are issued after
                # this sg's first k-chunk so the in-order Tensor queue runs its
                # transposes inside a DMA-wait bubble instead of stalling the
                # matmul stream (and only the last sg's finish is in the tail).
                acc = accp.tile([128, grp], F32)
                for k in range(KC):
                    hi_k = xh[k // KQ][:, k % KQ]   # [128, sg_t] fp16
                    lo_k = xl[k // KQ][:, k % KQ]   # [128, sg_t] fp8
                    for p in range(3):
                        w = (cwh, cwl, cw3)[p][:, k, :]
                        xs = (hi_k, hi_k, lo_k)[p]
                        first, last = (k == 0 and p == 0), (k == KC - 1 and p == 2)
                        nc.tensor.matmul(
                            acc[0:64], w, xs[:, 0:grp],
                            start=first, stop=last, tile_position=(0, 0),
                        )
                        nc.tensor.matmul(
                            acc[64:128], w, xs[:, grp:sg_t],
                            start=first, stop=last, tile_position=(0, 64),
                            skip_group_check=True,
                        )
                    if k == KQ - 1 and pending is not None:
                        finish_sg(*pending)
                        pending = None
                pending = (sg, acc)
            finish_sg(*pending)
    return nc


_CACHE = {}


def _get_compiled(tpc):
    if tpc not in _CACHE:
        nc = build_nc(tpc)
        nc.compile()
        _CACHE[tpc] = nc
    return _CACHE[tpc]


def _prep_shared(weight, bias):
    f16 = np.float16
    w = np.asarray(weight, np.float32)
    w_hi = w.astype(f16)
    w_lo = (w - w_hi.astype(np.float32)).astype(f16)
    w_3 = (w_hi.astype(np.float32) * (1.0 / 2048.0)).astype(f16)

    def wtile(a):  # [E, DIM] -> [128, KC, E]
        return np.ascontiguousarray(
            np.ascontiguousarray(a.T).reshape(KC, 128, E).transpose(1, 0, 2)
        )

    return {
        "wt_hi": wtile(w_hi),
        "wt_lo": wtile(w_lo),
        "wt_3": wtile(w_3),
        "bias_b": np.ascontiguousarray(
            np.broadcast_to(np.asarray(bias, np.float32), (128, E))
        ),
        "ident2": np.ascontiguousarray(
            np.tile(np.eye(64, dtype=np.float32), (2, 1))
        ),
    }


def prep_core_inputs(x, weight, bias, ncores=NCORES, sg_t=1024):
    f16 = np.float16
    e4 = ml_dtypes.float8_e4m3fn
    shared = _prep_shared(weight, bias)
    x = np.asarray(x, np.float32)
    tpc = x.shape[0] // ncores
    nsg = tpc // sg_t
    # whole-tensor transpose + casts once (not per core)
    xT = np.ascontiguousarray(x.T)           # [DIM, TOKENS]
    xhT = xT.astype(f16)
    xlT = ((xT - xhT.astype(np.float32)) * 2048.0).astype(e4)
    del xT
    in_maps = []
    for c in range(ncores):
        sl = slice(c * tpc, (c + 1) * tpc)
        # pack to [nsg, KC//KQ, 128, KQ, sg_t]: per (sg, kq, partition) the
        # [KQ, sg_t] block is one contiguous run in DRAM (8KB hi / 4KB lo)
        xh6 = xhT[:, sl].reshape(KC // KQ, KQ, 128, nsg, sg_t)
        xl6 = xlT[:, sl].reshape(KC // KQ, KQ, 128, nsg, sg_t)
        in_maps.append({
            "xhi": np.ascontiguousarray(xh6.transpose(3, 0, 2, 1, 4)),
            "xlo": np.ascontiguousarray(xl6.transpose(3, 0, 2, 1, 4)),
            **shared,
        })
    return in_maps


def unpack_outputs(res_list, tpc):
    ws, idxs = [], []
    for r in res_list:
        ev = np.asarray(r["ex_out"])  # [nsg, 128, nj, E]
        iv = np.asarray(r["i_out"])   # [nsg, 128, cols]
        nsg = ev.shape[0]
        nj = ev.shape[2]
        # token t = sg*sg_t + 128*j + p
        ev = ev.transpose(0, 2, 1, 3).reshape(tpc, E)
        iv = iv.reshape(nsg, 128, nj, TOPK).transpose(0, 2, 1, 3).reshape(tpc, TOPK)
        probs = ev / ev.sum(axis=-1, keepdims=True)
        wv = np.take_along_axis(probs, iv, axis=-1)
        ws.append(wv)
        idxs.append(iv)
    return (
        np.ascontiguousarray(np.concatenate(ws)).astype(np.float32),
        np.ascontiguousarray(np.concatenate(idxs)).astype(np.int32),
    )


def run(x, weight, bias, trace=False, **kwargs):
    x = np.asarray(x, np.float32)
    tpc = x.shape[0] // NCORES
    nc = _get_compiled(tpc)
    in_maps = prep_core_inputs(x, weight, bias)
    res = run_bass_kernel_spmd(nc, in_maps, list(range(NCORES)), trace=trace, **kwargs)
    w, i = unpack_outputs(res.results, tpc)
    return w, i, res


def kernel(x, weight, bias):
    w, i, _ = run(x, weight, bias, trace=False)
    return w, i

